# revision 1
# baseline (speedup 1.0000x reference)
"""Bass/Trainium2 kernel for BertSelfAttention with relation (graph) embeddings.

Reference computation (per batch b):
    q = (x @ Wq.T + bq)          k = x @ Wk.T + bk        v = x @ Wv.T + bv
    (split into H=16 heads of D=64)
    dp_k[0] = dp_v[0] = 0  (padding_idx)
    scores  = q·k/sqrt(D) + q·dp_k[g[q,k]] + mask
    probs   = softmax(scores)
    ctx     = probs @ v + sum_k probs * dp_v[g]
Sharding: data-parallel over batch (8 cores, one batch element each).

Kernel strategy (per core):
  - transpose X and W via PE-transpose; projections as fp32r matmuls in a
    transposed [feature, seq] layout so the contraction sits on partitions
  - scores[q,k] in PSUM, one 128-row q-tile per PSUM bank (q'=q/8 folded into
    the Q eviction, mask folded in as a rank-1 matmul)
  - relation score term  r_e[q]*(g==e)  via fused scalar_tensor_tensor on DVE
  - exp on ACT with fused row-sum accumulation (no max subtraction needed:
    |scores| <= ~few, fp32 exp is safe)
  - E transposed per 128x128 tile on PE (bf16), PV matmul with V stationary,
    relation value term via per-partition rank-1 updates, normalization folded
    into the final PSUM eviction
"""

import numpy as np

import concourse.bass as bass
import concourse.mybir as mybir
import concourse.tile as tile
from concourse import bacc
from concourse.bass_utils import run_bass_kernel_spmd
from concourse.masks import make_identity

F32 = mybir.dt.float32
F32R = mybir.dt.float32r
BF16 = mybir.dt.bfloat16
I32 = mybir.dt.int32
Alu = mybir.AluOpType
Act = mybir.ActivationFunctionType

B, S, HID, H, D = 8, 512, 1024, 16, 64
NCORES = 8
NQT = S // 128   # 4 q-tiles (also k-tiles) per sequence
NIT = HID // 128  # 8 tiles over the hidden dim
E_DTYPE = BF16    # dtype of exp(scores); BF16 halves DVE/PE cost of the PV path


def r32(ap):
    return ap.bitcast(F32R)


def build_module():
    nc = bacc.Bacc(
        "TRN2",
        target_bir_lowering=False,
        debug=False,
        enable_asserts=False,
        num_devices=NCORES,
    )
    x_in = nc.dram_tensor("x", [S, HID], F32, kind="ExternalInput").ap()
    mask_in = nc.dram_tensor("mask", [1, S], F32, kind="ExternalInput").ap()
    g_in = nc.dram_tensor("g", [S, S], I32, kind="ExternalInput").ap()
    wq_in = nc.dram_tensor("wq", [HID, HID], F32, kind="ExternalInput").ap()
    wk_in = nc.dram_tensor("wk", [HID, HID], F32, kind="ExternalInput").ap()
    wv_in = nc.dram_tensor("wv", [HID, HID], F32, kind="ExternalInput").ap()
    bq_in = nc.dram_tensor("bq", [HID], F32, kind="ExternalInput").ap()
    bk_in = nc.dram_tensor("bk", [HID], F32, kind="ExternalInput").ap()
    bv_in = nc.dram_tensor("bv", [HID], F32, kind="ExternalInput").ap()
    dpk_in = nc.dram_tensor("dpk", [3, D], F32, kind="ExternalInput").ap()
    dpv_in = nc.dram_tensor("dpv", [3, D], F32, kind="ExternalInput").ap()
    out_dram = nc.dram_tensor("out", [S, HID], F32, kind="ExternalOutput").ap()

    with tile.TileContext(nc) as tc:
        build_kernel(nc, tc, x_in, mask_in, g_in, wq_in, wk_in, wv_in,
                     bq_in, bk_in, bv_in, dpk_in, dpv_in, out_dram)
    nc.compile()
    return nc


def build_kernel(nc, tc, x_in, mask_in, g_in, wq_in, wk_in, wv_in,
                 bq_in, bk_in, bv_in, dpk_in, dpv_in, out_dram):
    from contextlib import ExitStack
    ctx = ExitStack()
    PP = ctx.enter_context(tc.tile_pool(name="persist", bufs=1))
    WP = ctx.enter_context(tc.tile_pool(name="wpool", bufs=4))
    WV = ctx.enter_context(tc.tile_pool(name="wvpool", bufs=2))
    WK = ctx.enter_context(tc.tile_pool(name="wrow", bufs=2))
    EW = ctx.enter_context(tc.tile_pool(name="ework", bufs=3))
    PS = ctx.enter_context(tc.tile_pool(name="ps_big", bufs=4, space="PSUM"))
    PT = ctx.enter_context(tc.tile_pool(name="ps_small", bufs=3, space="PSUM"))
    PV = ctx.enter_context(tc.tile_pool(name="ps_pv", bufs=1, space="PSUM"))

    # two engines share the PSUM->SBUF eviction load
    def evict(i, out, in_):
        if i % 2 == 0:
            nc.vector.tensor_copy(out, in_)
        else:
            nc.scalar.copy(out, in_)

    # ---- constants ----
    ident = PP.tile([128, 128], F32)
    make_identity(nc, ident[:])
    identb = PP.tile([128, 128], BF16)
    make_identity(nc, identb[:])
    ones_f32 = PP.tile([1, 128], F32)
    nc.vector.memset(ones_f32[:], 1.0)
    ones_row = PP.tile([1, 128], F32R)
    nc.vector.tensor_copy(ones_row[:], ones_f32[:])
    mask_sb = PP.tile([1, S], F32R)
    nc.gpsimd.dma_start(out=mask_sb[:], in_=mask_in)
    bq8 = PP.tile([128, NIT], F32)
    nc.sync.dma_start(out=bq8[:], in_=bq_in.rearrange("(t p) -> p t", p=128))
    nc.vector.tensor_scalar_mul(bq8[:], bq8[:], 0.125)
    bkc = PP.tile([128, NIT], F32)
    nc.sync.dma_start(out=bkc[:], in_=bk_in.rearrange("(t p) -> p t", p=128))
    bv_row = PP.tile([1, HID], F32R)
    nc.gpsimd.dma_start(out=bv_row[:], in_=bv_in.rearrange("(a o) -> a o", a=1))
    # 8*dp_k[1:3]^T duplicated in both partition halves so the rhs base
    # partition can match either head slot of a q-tile
    dpk8 = PP.tile([128, 2], F32R)
    nc.gpsimd.dma_start(out=dpk8[0:D, :], in_=dpk_in[1:3, :].rearrange("e d -> d e"))
    nc.gpsimd.dma_start(out=dpk8[D:128, :], in_=dpk_in[1:3, :].rearrange("e d -> d e"))
    nc.vector.tensor_scalar_mul(dpk8[:], dpk8[:], 8.0)
    dpv_rep = PP.tile([128, 2, D], F32)
    dpv_bcast = bass.AP(tensor=dpv_in.tensor, offset=D,
                        ap=[[0, 128], [D, 2], [1, D]])
    nc.gpsimd.dma_start(out=dpv_rep[:], in_=dpv_bcast)

    # ---- one-hot masks M_e = (g == e), bf16 ----
    m1 = PP.tile([128, NQT, S], BF16)
    m2 = PP.tile([128, NQT, S], BF16)
    for qt in range(NQT):
        gt = WK.tile([128, S], I32, tag="xrow")
        nc.sync.dma_start(out=gt[:], in_=g_in[128 * qt:128 * (qt + 1), :])
        nc.gpsimd.tensor_scalar(out=m1[:, qt, :], in0=gt[:], scalar1=1,
                                scalar2=None, op0=Alu.is_equal)
        nc.gpsimd.tensor_scalar(out=m2[:, qt, :], in0=gt[:], scalar1=2,
                                scalar2=None, op0=Alu.is_equal)

    # ---- X^T : [i, s] ----
    xt = PP.tile([128, NIT, S], F32R)
    nev = 0
    for st in range(NQT):
        xrow = WK.tile([128, HID], F32, tag="xrow")
        nc.sync.dma_start(out=xrow[:], in_=x_in[128 * st:128 * (st + 1), :])
        for it in range(NIT):
            pst = PT.tile([128, 128], F32, tag="pt")
            nc.tensor.transpose(pst[:], xrow[:, 128 * it:128 * (it + 1)], ident[:])
            evict(nev, xt[:, it, 128 * st:128 * (st + 1)], pst[:])
            nev += 1

    # ---- W^T + projections ----
    qt_sb = PP.tile([128, NIT, S], F32R)  # Q'^T = (X Wq^T + bq)^T / 8
    kt_sb = PP.tile([128, NIT, S], F32R)  # K^T
    vb = PP.tile([128, NQT, H, D], BF16)  # V natural, by (k-tile, head, d)

    # Q^T and K^T: for each output o-tile t, transpose W's row-block t
    # into a small rolling tile, then contract against X^T over all i-tiles.
    for wi, (w_in, b_col, o_sb, scale) in enumerate((
            (wq_in, bq8, qt_sb, 0.125),
            (wk_in, bkc, kt_sb, 1.0))):
        for t in range(NIT):
            wrow = WK.tile([128, HID], F32, tag="wrow")
            nc.sync.dma_start(out=wrow[:], in_=w_in[128 * t:128 * (t + 1), :])
            wtile = WP.tile([128, NIT, 128], F32R, tag="wt")
            for it in range(NIT):
                pst = PT.tile([128, 128], F32, tag="pt")
                nc.tensor.transpose(pst[:], wrow[:, 128 * it:128 * (it + 1)], ident[:])
                evict(nev, wtile[:, it, :], pst[:])
                nev += 1
            ps = PS.tile([128, S], F32, tag="psbig")
            for it in range(NIT):
                nc.tensor.matmul(ps[:], r32(wtile[:, it, :]), r32(xt[:, it, :]),
                                 start=(it == 0), stop=(it == NIT - 1))
            nc.scalar.activation(o_sb[:, t, :], ps[:], Act.Identity,
                                 bias=b_col[:, t:t + 1], scale=scale)

    # V (natural layout): per 512-wide output chunk, transpose 4 row-blocks of
    # Wv, then produce the 4 s-tiles of that chunk.
    for oc in range(2):
        wtv = WV.tile([128, NIT, 512], F32R, tag="wtv")
        for tb in range(4):
            wrow = WK.tile([128, HID], F32, tag="wrow")
            nc.sync.dma_start(
                out=wrow[:],
                in_=wv_in[512 * oc + 128 * tb:512 * oc + 128 * (tb + 1), :])
            for it in range(NIT):
                pst = PT.tile([128, 128], F32, tag="pt")
                nc.tensor.transpose(pst[:], wrow[:, 128 * it:128 * (it + 1)], ident[:])
                evict(nev, wtv[:, it, 128 * tb:128 * (tb + 1)], pst[:])
                nev += 1
        for st in range(NQT):
            ps = PS.tile([128, S], F32, tag="psbig")
            for it in range(NIT):
                nc.tensor.matmul(ps[:], r32(xt[:, it, 128 * st:128 * (st + 1)]),
                                 r32(wtv[:, it, :]),
                                 start=(it == 0), stop=False)
            nc.tensor.matmul(ps[:], r32(ones_row[:]),
                             r32(bv_row[:, 512 * oc:512 * (oc + 1)]),
                             start=False, stop=True)
            nc.vector.tensor_copy(
                vb[:, st, 8 * oc:8 * (oc + 1), :],
                ps[:].rearrange("p (h d) -> p h d", d=D))

    # ---- attention, one head at a time ----
    import os
    n_heads = int(os.environ.get("KERNEL_NHEADS", str(H)))
    osb = PP.tile([128, NQT, HID], F32)
    if n_heads < H:
        nc.gpsimd.memset(osb[:], 0.0)
        if os.environ.get("KERNEL_DUMP_PROJ") == "1":
            nc.vector.tensor_copy(osb[:, 0, 0:S], qt_sb[:, 0, :])
            nc.vector.tensor_copy(osb[:, 0, S:2 * S], qt_sb[:, 1, :])
            nc.vector.tensor_copy(osb[:, 1, 0:S], kt_sb[:, 0, :])
            nc.vector.tensor_copy(osb[:, 1, S:2 * S], kt_sb[:, 1, :])
            nc.vector.tensor_copy(osb[:, 2, :], vb[:, 0, :, :].rearrange("p h d -> p (h d)"))
            nc.vector.tensor_copy(osb[:, 3, 0:S], xt[:, 0, :])
    stage = int(os.environ.get("KERNEL_HEAD_STAGE", "8"))
    for h in range(n_heads):
        t, po = h // 2, D * (h % 2)
        q_ap = [qt_sb[po:po + D, t, 128 * qt:128 * (qt + 1)] for qt in range(NQT)]
        k_ap = kt_sb[po:po + D, t, :]

        psS = []
        for qt in range(NQT):
            ps = PS.tile([128, S], F32, tag="psbig")
            nc.tensor.matmul(ps[:], r32(q_ap[qt]), r32(k_ap), start=True, stop=False)
            nc.tensor.matmul(ps[:], r32(ones_row[:]), r32(mask_sb[:]),
                             start=False, stop=True)
            psS.append(ps)

        rcols = EW.tile([128, NQT, 2], F32, tag="rcols")
        for qt in range(NQT if stage >= 2 else 0):
            psr = PT.tile([128, 2], F32, tag="pt")
            nc.tensor.matmul(psr[:], q_ap[qt], dpk8[po:po + D, :],
                             start=True, stop=True)
            nc.scalar.copy(rcols[:, qt, :], psr[:])

        for qt in range(NQT if stage >= 3 else 0):
            nc.vector.scalar_tensor_tensor(
                out=psS[qt][:], in0=m1[:, qt, :], scalar=rcols[:, qt, 0:1],
                in1=psS[qt][:], op0=Alu.mult, op1=Alu.add)
            nc.vector.scalar_tensor_tensor(
                out=psS[qt][:], in0=m2[:, qt, :], scalar=rcols[:, qt, 1:2],
                in1=psS[qt][:], op0=Alu.mult, op1=Alu.add)

        esb = EW.tile([128, NQT, S], E_DTYPE, tag="esb")
        ssum = EW.tile([128, NQT], F32, tag="ssum")
        rsum = EW.tile([128, NQT], F32, tag="rsum")
        for qt in range(NQT if stage >= 4 else 0):
            nc.scalar.activation(esb[:, qt, :], psS[qt][:], Act.Exp,
                                 accum_out=ssum[:, qt:qt + 1])
            nc.vector.reciprocal(rsum[:, qt:qt + 1], ssum[:, qt:qt + 1])

        # p_e[q] = sum_k E * M_e  (unnormalized)
        p12 = EW.tile([128, NQT, 2], F32, tag="p12")
        # p_e = sum_k E*M_e via InstTensorScalarPtr with accumulate
        # (tensor_tensor_reduce is a custom DVE op and crashes on this stack)
        pscr = EW.tile([128, S], E_DTYPE, tag="pscr")
        for qt in range(NQT if stage >= 5 else 0):
            nc.vector.scalar_tensor_tensor(
                out=pscr[:], in0=m1[:, qt, :], scalar=1.0, in1=esb[:, qt, :],
                op0=Alu.mult, op1=Alu.mult, accum_out=p12[:, qt, 0:1])
            nc.vector.scalar_tensor_tensor(
                out=pscr[:], in0=m2[:, qt, :], scalar=1.0, in1=esb[:, qt, :],
                op0=Alu.mult, op1=Alu.mult, accum_out=p12[:, qt, 1:2])

        # E^T
        etb = EW.tile([128, NQT, S], E_DTYPE, tag="etb")
        for qt in range(NQT if stage >= 6 else 0):
            for kt in range(NQT):
                pst = PT.tile([128, 128], E_DTYPE, tag="pt")
                nc.tensor.transpose(pst[:], esb[:, qt, 128 * kt:128 * (kt + 1)],
                                    identb[:])
                evict(nev, etb[:, kt, 128 * qt:128 * (qt + 1)], pst[:])
                nev += 1

        # ctx^T = V^T E^T  -> [d, q]
        psC = PV.tile([D, S], F32, tag="psc")
        for kt in range(NQT if stage >= 7 else 0):
            nc.tensor.matmul(psC[:], vb[:, kt, h, :], etb[:, kt, :],
                             start=(kt == 0), stop=(kt == NQT - 1))
        cts = EW.tile([D, S], F32, tag="cts")
        if stage >= 7:
            nc.vector.tensor_copy(cts[:], psC[:])

        # transpose back, add relation-value term, normalize
        for qt in range(NQT if stage >= 8 else 0):
            psX = PT.tile([128, D], F32, tag="pt")
            nc.tensor.transpose(psX[:], cts[:, 128 * qt:128 * (qt + 1)],
                                ident[0:D, 0:D])
            nc.vector.scalar_tensor_tensor(
                out=psX[:], in0=dpv_rep[:, 0, :], scalar=p12[:, qt, 0:1],
                in1=psX[:], op0=Alu.mult, op1=Alu.add)
            nc.vector.scalar_tensor_tensor(
                out=psX[:], in0=dpv_rep[:, 1, :], scalar=p12[:, qt, 1:2],
                in1=psX[:], op0=Alu.mult, op1=Alu.add)
            nc.vector.tensor_scalar(
                out=osb[:, qt, D * h:D * (h + 1)], in0=psX[:],
                scalar1=rsum[:, qt:qt + 1], scalar2=None, op0=Alu.mult)

    if stage < 8 and n_heads > 0:
        if stage >= 4:
            nc.vector.tensor_copy(osb[:, 0, 0:S], esb[:, 0, :])
        else:
            nc.vector.tensor_copy(osb[:, 0, 0:S], psS[0][:])
        if stage >= 6:
            nc.vector.tensor_copy(osb[:, 1, 0:S], etb[:, 0, :])
        if stage >= 7:
            nc.vector.tensor_copy(osb[:, 2, 0:S], cts[0:D, :].rearrange("d s -> d s"))
    nc.sync.dma_start(out=out_dram.rearrange("(qt p) o -> p qt o", p=128),
                      in_=osb[:])
    ctx.close()


_NC = None


def _get_module():
    global _NC
    if _NC is None:
        _NC = build_module()
    return _NC


def make_in_maps(hidden_states, attention_mask, graph_emb, Wq, bq, Wk, bk,
                 Wv, bv, dp_k, dp_v):
    hidden_states = np.ascontiguousarray(hidden_states, dtype=np.float32)
    attention_mask = np.ascontiguousarray(attention_mask, dtype=np.float32)
    graph_emb = np.ascontiguousarray(graph_emb, dtype=np.int32)
    shared = {
        "wq": np.ascontiguousarray(Wq, dtype=np.float32),
        "wk": np.ascontiguousarray(Wk, dtype=np.float32),
        "wv": np.ascontiguousarray(Wv, dtype=np.float32),
        "bq": np.ascontiguousarray(bq, dtype=np.float32),
        "bk": np.ascontiguousarray(bk, dtype=np.float32),
        "bv": np.ascontiguousarray(bv, dtype=np.float32),
        "dpk": np.ascontiguousarray(dp_k, dtype=np.float32),
        "dpv": np.ascontiguousarray(dp_v, dtype=np.float32),
    }
    in_maps = []
    for c in range(NCORES):
        in_maps.append({
            "x": hidden_states[c],
            "mask": attention_mask[c].reshape(1, S),
            "g": graph_emb[c],
            **shared,
        })
    return in_maps


def kernel(**inputs):
    nc = _get_module()
    in_maps = make_in_maps(**inputs)
    res = run_bass_kernel_spmd(nc, in_maps, list(range(NCORES)))
    out = np.stack([res.results[c]["out"] for c in range(NCORES)], axis=0)
    return out.astype(np.float32)


if __name__ == "__main__":
    rng = np.random.default_rng(0)
    inputs = {
        "hidden_states": rng.standard_normal((B, S, HID)).astype(np.float32),
        "attention_mask": np.zeros((B, 1, 1, S), np.float32),
        "graph_emb": rng.integers(0, 3, (B, S, S)).astype(np.int32),
        "Wq": (rng.standard_normal((HID, HID)) * 0.02).astype(np.float32),
        "bq": np.zeros(HID, np.float32),
        "Wk": (rng.standard_normal((HID, HID)) * 0.02).astype(np.float32),
        "bk": np.zeros(HID, np.float32),
        "Wv": (rng.standard_normal((HID, HID)) * 0.02).astype(np.float32),
        "bv": np.zeros(HID, np.float32),
        "dp_k": (rng.standard_normal((3, D)) * 0.02).astype(np.float32),
        "dp_v": (rng.standard_normal((3, D)) * 0.02).astype(np.float32),
    }
    out = kernel(**inputs)
    print("out", out.shape, out.dtype, float(np.abs(out).max()))



# revision 19
# speedup vs baseline: 2.5085x; 2.5085x over previous
"""Bass/Trainium2 kernel for BertSelfAttention with relation (graph) embeddings.

Reference computation (per batch b):
    q = x @ Wq.T + bq        k = x @ Wk.T + bk        v = x @ Wv.T + bv
    (split into H=16 heads of D=64)
    dp_k[0] = dp_v[0] = 0  (padding_idx)
    scores  = q.k/sqrt(D) + q.dp_k[g[q,k]] + mask
    probs   = softmax(scores)
    ctx     = probs @ v + sum_k probs * dp_v[g]

Sharding: data-parallel over batch (8 cores, one batch element each).

Design notes (v2):
  - everything bf16 on the PE: fp32 matmuls run 3-4x slower (fp32_mode=HIGH)
  - W^T and X^T are prepared host-side; no PE transposes for projections
  - relation score term r_e[q]*M_e[q,k] is a diag(r_e) @ M_e matmul on the
    PE (diag built with one tensor_scalar off the identity), accumulated
    straight into the scores PSUM - no DVE scalar_tensor_tensor on PSUM
  - E^T via DMA-transpose (xbar), SBUF->SBUF, per 128x128 tile - frees PE
  - relation value term folded into the PV matmul: ctx^T += dpv_e-bcast.T @
    (M_e^T . E^T); softmax denominator Z rides along as a ones column of V
  - normalization folded into the final PSUM eviction via activation scale
"""

import numpy as np
import ml_dtypes

import concourse.bass as bass
import concourse.mybir as mybir
import concourse.tile as tile
from concourse import bacc
from concourse.bass_utils import run_bass_kernel_spmd
from concourse.masks import make_identity

F32 = mybir.dt.float32
BF16 = mybir.dt.bfloat16
I32 = mybir.dt.int32
Alu = mybir.AluOpType
Act = mybir.ActivationFunctionType

B, S, HID, H, D = 8, 512, 1024, 16, 64
NCORES = 8
NQT = S // 128    # 4 q-tiles (also k-tiles) per sequence
NIT = HID // 128  # 8 tiles over the hidden dim


def build_module(with_mask, with_bias):
    nc = bacc.Bacc(
        "TRN2",
        target_bir_lowering=False,
        debug=False,
        enable_asserts=False,
        num_devices=NCORES,
    )
    xt_in = nc.dram_tensor("xt", [HID, S], BF16, kind="ExternalInput").ap()
    g_in = nc.dram_tensor("g", [S, S], I32, kind="ExternalInput").ap()
    gt_in = nc.dram_tensor("gt", [S, S], I32, kind="ExternalInput").ap()
    wqt_in = nc.dram_tensor("wqt", [HID, HID], BF16, kind="ExternalInput").ap()
    wkt_in = nc.dram_tensor("wkt", [HID, HID], BF16, kind="ExternalInput").ap()
    wvt_in = nc.dram_tensor("wvt", [HID, HID], BF16, kind="ExternalInput").ap()
    dpkbd_in = nc.dram_tensor("dpkbd", [128, 4], BF16, kind="ExternalInput").ap()
    dpv_in = nc.dram_tensor("dpv", [2, D], BF16, kind="ExternalInput").ap()
    mask_in = bias_in = None
    if with_mask:
        mask_in = nc.dram_tensor("mask", [1, S], BF16, kind="ExternalInput").ap()
    if with_bias:
        # bq/8, bk in column layout [128, NIT]; bv natural row [1, HID]
        bias_in = (
            nc.dram_tensor("bqc", [128, NIT], F32, kind="ExternalInput").ap(),
            nc.dram_tensor("bkc", [128, NIT], F32, kind="ExternalInput").ap(),
            nc.dram_tensor("bvr", [1, HID], BF16, kind="ExternalInput").ap(),
        )
    out_dram = nc.dram_tensor("out", [S, HID], BF16, kind="ExternalOutput").ap()

    with tile.TileContext(nc) as tc:
        build_kernel(nc, tc, xt_in, g_in, gt_in, wqt_in, wkt_in, wvt_in,
                     dpkbd_in, dpv_in, mask_in, bias_in, out_dram)
    nc.compile()
    return nc


def build_kernel(nc, tc, xt_in, g_in, gt_in, wqt_in, wkt_in, wvt_in,
                 dpkbd_in, dpv_in, mask_in, bias_in, out_dram):
    from contextlib import ExitStack
    ctx = ExitStack()
    PP = ctx.enter_context(tc.tile_pool(name="persist", bufs=1))
    WP = ctx.enter_context(tc.tile_pool(name="wpool", bufs=2))
    GP = ctx.enter_context(tc.tile_pool(name="gpool", bufs=2))
    EB = ctx.enter_context(tc.tile_pool(name="epool", bufs=2))
    TB = ctx.enter_context(tc.tile_pool(name="etpool", bufs=2))
    AB = ctx.enter_context(tc.tile_pool(name="apool", bufs=2))
    DG = ctx.enter_context(tc.tile_pool(name="dgpool", bufs=4))
    CT = ctx.enter_context(tc.tile_pool(name="ctpool", bufs=2))
    # PSUM (8 banks): scores 3 + E^T-stage 2 + ctx 1 + out-transpose 2.
    # The projection pool is scoped and released before ET/PC/PX open.
    PS = ctx.enter_context(tc.tile_pool(name="ps_s", bufs=3, space="PSUM"))

    # ---- constants ----
    identb = PP.tile([128, 128], BF16)
    make_identity(nc, identb[:])
    allones_i32 = PP.tile([128, 1], I32)
    nc.vector.memset(allones_i32[:], -1)
    dpkbd = PP.tile([128, 4], BF16)
    nc.sync.dma_start(out=dpkbd[:], in_=dpkbd_in)
    # dpv rows broadcast to all 128 partitions: [128, 2, D]
    dpvbc = PP.tile([128, 2, D], BF16)
    dpv_b = bass.AP(tensor=dpv_in.tensor, offset=0, ap=[[0, 128], [D, 2], [1, D]])
    nc.sync.dma_start(out=dpvbc[:], in_=dpv_b)
    if mask_in is not None:
        mask_sb = PP.tile([1, S], BF16)
        nc.sync.dma_start(out=mask_sb[:], in_=mask_in)
        ones_row = PP.tile([1, 128], BF16)
        nc.vector.memset(ones_row[:], 1.0)
    if bias_in is not None:
        bqc = PP.tile([128, NIT], F32)
        nc.sync.dma_start(out=bqc[:], in_=bias_in[0])
        bkc = PP.tile([128, NIT], F32)
        nc.sync.dma_start(out=bkc[:], in_=bias_in[1])
        bvr = PP.tile([1, HID], BF16)
        nc.sync.dma_start(out=bvr[:], in_=bias_in[2])
        ones_rowv = PP.tile([1, 128], BF16)
        nc.vector.memset(ones_rowv[:], 1.0)

    # ---- X^T ----
    xt = PP.tile([128, NIT, S], BF16)
    nc.sync.dma_start(out=xt[:], in_=xt_in.rearrange("(t p) s -> p t s", p=128))

    # ---- masks: natural in bf16 (matmul rhs), transposed as uint16
    # bit-masks 0xFFFF/0x0000 (for bitwise-AND masking of E^T) ----
    U16 = mybir.dt.uint16
    m1 = PP.tile([128, NQT, S], BF16)
    m2 = PP.tile([128, NQT, S], BF16)
    m1t = PP.tile([128, NQT, S], U16)
    m2t = PP.tile([128, NQT, S], U16)
    for qt in range(NQT):
        gt_ = GP.tile([128, S], I32, tag="g")
        nc.sync.dma_start(out=gt_[:], in_=g_in[128 * qt:128 * (qt + 1), :])
        nc.vector.tensor_scalar(out=m1[:, qt, :], in0=gt_[:], scalar1=1,
                                scalar2=None, op0=Alu.is_equal)
        nc.vector.tensor_scalar(out=m2[:, qt, :], in0=gt_[:], scalar1=2,
                                scalar2=None, op0=Alu.is_equal)
    for kt in range(NQT):
        gt_ = GP.tile([128, S], I32, tag="g")
        nc.sync.dma_start(out=gt_[:], in_=gt_in[128 * kt:128 * (kt + 1), :])
        nc.vector.tensor_scalar(out=m1t[:, kt, :], in0=gt_[:], scalar1=1,
                                scalar2=65535, op0=Alu.is_equal, op1=Alu.mult)
        nc.vector.tensor_scalar(out=m2t[:, kt, :], in0=gt_[:], scalar1=2,
                                scalar2=65535, op0=Alu.is_equal, op1=Alu.mult)

    # ---- projections (scoped PSUM pool, released before head phases) ----
    qt_sb = PP.tile([128, NIT, S], BF16)   # Q^T/8: [feature, seq]
    kt_sb = PP.tile([128, NIT, S], BF16)   # K^T
    vb = PP.tile([128, NQT, H, D + 1], BF16)  # V natural + ones column

    PBIG = tc.alloc_tile_pool(name="ps_proj", bufs=2, space="PSUM")
    for wi, (w_in, o_sb, scale) in enumerate(((wqt_in, qt_sb, 0.125),
                                              (wkt_in, kt_sb, 1.0))):
        for t in range(NIT):
            wt = WP.tile([128, NIT, 128], BF16, tag="wqk")
            nc.sync.dma_start(
                out=wt[:],
                in_=w_in[:, 128 * t:128 * (t + 1)].rearrange(
                    "(it p) o -> p it o", p=128))
            ps = PBIG.tile([128, S], F32, tag="psbig")
            for it in range(NIT):
                nc.tensor.matmul(ps[:], wt[:, it, :], xt[:, it, :],
                                 start=(it == 0), stop=(it == NIT - 1))
            if bias_in is not None:
                bcol = (bqc if wi == 0 else bkc)[:, t:t + 1]
                nc.scalar.activation(o_sb[:, t, :], ps[:], Act.Identity,
                                     bias=bcol, scale=scale)
            else:
                nc.scalar.activation(o_sb[:, t, :], ps[:], Act.Identity,
                                     scale=scale)

    # rcols: r_e[q] for all (t, qt) pairs in one PSUM bank
    # layout [128, NIT, NQT, 4]; cols (2*(h%2)+e-1)
    psr = PS.tile([128, NIT, NQT, 4], F32, tag="ps_s")
    for t in range(NIT):
        for qt in range(NQT):
            nc.tensor.matmul(psr[:, t, qt, :],
                             qt_sb[:, t, 128 * qt:128 * (qt + 1)], dpkbd[:],
                             start=(t == 0 and qt == 0),
                             stop=(t == NIT - 1 and qt == NQT - 1))
    rcol = PP.tile([128, NIT, NQT, 4], F32)
    nc.vector.tensor_copy(rcol[:], psr[:])

    # V in natural layout [s, o] via lhsT = x^T
    for oc in range(2):
        wt = WP.tile([128, NIT, 512], BF16, tag="wv")
        nc.sync.dma_start(
            out=wt[:],
            in_=wvt_in[:, 512 * oc:512 * (oc + 1)].rearrange(
                "(it p) o -> p it o", p=128))
        for st in range(NQT):
            ps = PBIG.tile([128, 512], F32, tag="psbig")
            for it in range(NIT):
                nc.tensor.matmul(ps[:], xt[:, it, 128 * st:128 * (st + 1)],
                                 wt[:, it, :],
                                 start=(it == 0),
                                 stop=(it == NIT - 1 and bias_in is None))
            if bias_in is not None:
                nc.tensor.matmul(ps[:], ones_rowv[:],
                                 bvr[:, 512 * oc:512 * (oc + 1)],
                                 start=False, stop=True)
            nc.vector.tensor_copy(
                vb[:, st, 8 * oc:8 * (oc + 1), 0:D],
                ps[:].rearrange("p (h d) -> p h d", d=D))
    # ones column for the Z row of ctx^T
    nc.gpsimd.memset(vb[:, :, :, D:D + 1], 1.0)
    PBIG.release()
    ET = ctx.enter_context(tc.tile_pool(name="ps_et", bufs=2, space="PSUM"))
    PC = ctx.enter_context(tc.tile_pool(name="ps_c", bufs=1, space="PSUM"))
    PX = ctx.enter_context(tc.tile_pool(name="ps_x", bufs=2, space="PSUM"))

    # ---- attention: software-pipelined over heads ----
    # Stages (lagged so every PE instruction's inputs are >= 1 head old):
    #   S(h): scores MMs + exp        T(h): E^T PE-transposes + A-products
    #   P(h): PV/relval MMs + evict   O(h): out-transposes + normalize
    osb = PP.tile([128, NQT, HID], BF16)
    state = {}

    def phase_scores(h):
        t, po = h // 2, D * (h % 2)
        e0 = 2 * (h % 2)  # rcol column base for this head
        esb = EB.tile([128, NQT, S], BF16, tag="esb")
        for qt in range(NQT):
            ps = PS.tile([128, S], F32, tag="ps_s")
            nc.tensor.matmul(ps[:], qt_sb[po:po + D, t, 128 * qt:128 * (qt + 1)],
                             kt_sb[po:po + D, t, :], start=True, stop=False)
            for e in range(2):
                dg = DG.tile([128, 128], BF16, tag="dg")
                nc.vector.tensor_scalar(
                    out=dg[:], in0=identb[:],
                    scalar1=rcol[:, t, qt, e0 + e:e0 + e + 1],
                    scalar2=None, op0=Alu.mult)
                last = (e == 1) and mask_in is None
                nc.tensor.matmul(ps[:], dg[:], (m1 if e == 0 else m2)[:, qt, :],
                                 start=False, stop=last)
            if mask_in is not None:
                nc.tensor.matmul(ps[:], ones_row[:], mask_sb[:],
                                 start=False, stop=True)
            nc.scalar.activation(esb[:, qt, :], ps[:], Act.Exp)
        state[h] = {"esb": esb}

    def phase_transpose(h):
        st = state[h]
        esb = st["esb"]
        # 4 PE transposes share one PSUM tile (start=True only clears
        # has_written, data of other quarters survives), one ACT eviction
        etb = TB.tile([128, NQT, S], BF16, tag="etb")
        for qt in range(NQT):
            pst = ET.tile([128, NQT, 128], BF16, tag="et")
            for kt in range(NQT):
                nc.tensor.transpose(pst[:, kt, :],
                                    esb[:, qt, 128 * kt:128 * (kt + 1)],
                                    identb[:])
            nc.scalar.copy(etb[:, :, 128 * qt:128 * (qt + 1)], pst[:])
        # A_e^T = M_e^T . E^T as bitwise AND of E^T bits with the 0xFFFF
        # bit-masks, processed as int32 pairs (half the element count)
        a1t = AB.tile([128, NQT, S], BF16, tag="a1")
        a2t = AB.tile([128, NQT, S], BF16, tag="a2")
        for at, mt in ((a1t, m1t), (a2t, m2t)):
            for kt in range(0, NQT, 2):
                nc.vector.scalar_tensor_tensor(
                    out=at[:, kt:kt + 2, :].bitcast(I32),
                    in0=mt[:, kt:kt + 2, :].bitcast(I32),
                    scalar=allones_i32[:, 0:1],
                    in1=etb[:, kt:kt + 2, :].bitcast(I32),
                    op0=Alu.bitwise_and, op1=Alu.bitwise_and)
        st.update(etb=etb, a1t=a1t, a2t=a2t)

    def phase_pv(h):
        st = state[h]
        etb, a1t, a2t = st["etb"], st["a1t"], st["a2t"]
        # ctx^T (+Z row) = [V|1]^T.T @ E^T + sum_e dpv_e-bcast.T @ A_e^T
        psC = PC.tile([D + 1, S], F32, tag="psc")
        for kt in range(NQT):
            nc.tensor.matmul(psC[:], vb[:, kt, h, :], etb[:, kt, :],
                             start=(kt == 0), stop=False)
        for e in range(2):
            at = a1t if e == 0 else a2t
            for kt in range(NQT):
                nc.tensor.matmul(psC[0:D, :], dpvbc[:, e, :], at[:, kt, :],
                                 start=False,
                                 stop=(e == 1 and kt == NQT - 1))
        cts = CT.tile([D + 1, S], BF16, tag="cts")
        nc.vector.tensor_copy(cts[:], psC[:])
        st["cts"] = cts

    def phase_out(h):
        cts = state.pop(h)["cts"]
        for qt in range(NQT):
            psX = PX.tile([128, D + 1], BF16, tag="psx")
            nc.tensor.transpose(psX[:], cts[:, 128 * qt:128 * (qt + 1)],
                                identb[0:D + 1, 0:D + 1])
            rz = DG.tile([128, 1], F32, tag="rz")
            nc.vector.reciprocal(rz[:], psX[:, D:D + 1])
            nc.scalar.activation(osb[:, qt, D * h:D * (h + 1)], psX[:, 0:D],
                                 Act.Identity, scale=rz[:, 0:1])

    for i in range(H + 3):
        if i < H:
            phase_scores(i)
        if 0 <= i - 1 < H:
            phase_transpose(i - 1)
        if 0 <= i - 2 < H:
            phase_pv(i - 2)
        if 0 <= i - 3 < H:
            phase_out(i - 3)

    nc.sync.dma_start(out=out_dram.rearrange("(qt p) o -> p qt o", p=128),
                      in_=osb[:])
    ctx.close()


_NC = None
_NC_KEY = None


def _get_module(with_mask=False, with_bias=False):
    global _NC, _NC_KEY
    key = (with_mask, with_bias)
    if _NC is None or _NC_KEY != key:
        _NC = build_module(with_mask, with_bias)
        _NC_KEY = key
    return _NC


def make_in_maps(hidden_states, attention_mask, graph_emb, Wq, bq, Wk, bk,
                 Wv, bv, dp_k, dp_v):
    with_mask = bool(np.any(np.asarray(attention_mask)))
    with_bias = bool(np.any(bq) or np.any(bk) or np.any(bv))

    bf = ml_dtypes.bfloat16
    x = np.ascontiguousarray(np.asarray(hidden_states), dtype=np.float32)
    g = np.ascontiguousarray(np.asarray(graph_emb), dtype=np.int32)

    # 8 * dp_k[1:3]^T replicated in both 64-row halves as block-diagonal
    # [128, 4]: rows 0:64 cols 0:2 = head-even, rows 64:128 cols 2:4 = head-odd
    dpk = np.asarray(dp_k, dtype=np.float32)
    dpkbd = np.zeros((128, 4), dtype=np.float32)
    dpkbd[0:D, 0:2] = 8.0 * dpk[1:3].T
    dpkbd[D:128, 2:4] = 8.0 * dpk[1:3].T

    shared = {
        "wqt": np.ascontiguousarray(np.asarray(Wq, dtype=np.float32).T).astype(bf),
        "wkt": np.ascontiguousarray(np.asarray(Wk, dtype=np.float32).T).astype(bf),
        "wvt": np.ascontiguousarray(np.asarray(Wv, dtype=np.float32).T).astype(bf),
        "dpkbd": dpkbd.astype(bf),
        "dpv": np.asarray(dp_v, dtype=np.float32)[1:3].astype(bf),
    }
    if with_mask:
        shared_mask = np.asarray(attention_mask, dtype=np.float32)
    if with_bias:
        shared["bqc"] = np.ascontiguousarray(
            (np.asarray(bq, dtype=np.float32) / 8.0).reshape(NIT, 128).T)
        shared["bkc"] = np.ascontiguousarray(
            np.asarray(bk, dtype=np.float32).reshape(NIT, 128).T)
        shared["bvr"] = np.asarray(bv, dtype=np.float32).reshape(1, HID).astype(bf)

    in_maps = []
    for c in range(NCORES):
        m = {
            "xt": np.ascontiguousarray(x[c].T).astype(bf),
            "g": g[c],
            "gt": np.ascontiguousarray(g[c].T),
            **shared,
        }
        if with_mask:
            m["mask"] = shared_mask[c].reshape(1, S).astype(bf)
        in_maps.append(m)
    return in_maps, with_mask, with_bias


def kernel(**inputs):
    in_maps, with_mask, with_bias = make_in_maps(**inputs)
    nc = _get_module(with_mask, with_bias)
    res = run_bass_kernel_spmd(nc, in_maps, list(range(NCORES)))
    out = np.stack([res.results[c]["out"] for c in range(NCORES)], axis=0)
    return out.astype(np.float32)


if __name__ == "__main__":
    rng = np.random.default_rng(0)
    inputs = {
        "hidden_states": rng.standard_normal((B, S, HID)).astype(np.float32),
        "attention_mask": np.zeros((B, 1, 1, S), np.float32),
        "graph_emb": rng.integers(0, 3, (B, S, S)).astype(np.int32),
        "Wq": (rng.standard_normal((HID, HID)) * 0.02).astype(np.float32),
        "bq": np.zeros(HID, np.float32),
        "Wk": (rng.standard_normal((HID, HID)) * 0.02).astype(np.float32),
        "bk": np.zeros(HID, np.float32),
        "Wv": (rng.standard_normal((HID, HID)) * 0.02).astype(np.float32),
        "bv": np.zeros(HID, np.float32),
        "dp_k": (rng.standard_normal((3, D)) * 0.02).astype(np.float32),
        "dp_v": (rng.standard_normal((3, D)) * 0.02).astype(np.float32),
    }
    out = kernel(**inputs)
    print("out", out.shape, out.dtype, float(np.abs(out).max()))
